# revision 3
# baseline (speedup 1.0000x reference)
"""ChebGraphConv (K=3) Trainium2 kernel.

y = x@(W0-W2) - (A@x)@W1 + 2*A@((A@x)@W2) + bias

computed per (b,t) slice as:
  P0 = X@W02 + bias ; P1 = X@W1 ; P2 = X@W2   (projections, from X^T hi/lo bf16)
  Q = A@P2 ; M = P1 - 2Q ; S = A@M ; y = P0 - S  (spmms, bf16 A^T stationary)

Data parallel over B: core i handles x[i] (T=12 slices), slices processed in
groups of G=6 so the spmm moving operand is [128, 384].
"""

import numpy as np
import ml_dtypes

import concourse.bacc as bacc
import concourse.mybir as mybir
import concourse.tile as tile
from concourse import bass_utils

BF16 = ml_dtypes.bfloat16

B, T, N, C = 8, 12, 2048, 64
NB = N // 128          # 16 node blocks
G = 6                  # slices per group
NG = T // G            # 2 groups
GW = G * C             # 384: group free width

_NC_CACHE = {}


def _build_nc():
    if "nc" in _NC_CACHE:
        return _NC_CACHE["nc"]
    f32 = mybir.dt.float32
    bf16 = mybir.dt.bfloat16

    nc = bacc.Bacc("TRN2", target_bir_lowering=False, debug=False,
                   enable_asserts=False, num_devices=8)

    at_d = nc.dram_tensor("at", [NB, 128, N], bf16, kind="ExternalInput")
    xs_d = nc.dram_tensor("xs", [T, 128, N], bf16, kind="ExternalInput")
    wa_d = nc.dram_tensor("wa", [128, 3 * C], bf16, kind="ExternalInput")
    wb_d = nc.dram_tensor("wb", [128, C], bf16, kind="ExternalInput")
    bias_d = nc.dram_tensor("biasb", [128, C], f32, kind="ExternalInput")
    y_d = nc.dram_tensor("y", [T, N, C], f32, kind="ExternalOutput")

    with tile.TileContext(nc) as tc:
        with (
            tc.tile_pool(name="const", bufs=1) as constp,
            tc.tile_pool(name="atp", bufs=1) as atp,
            tc.tile_pool(name="xsp", bufs=1) as xsp,
            tc.tile_pool(name="p0p", bufs=1) as p0p,
            tc.tile_pool(name="grp", bufs=1) as grp,
            tc.tile_pool(name="mtmpp", bufs=2) as mtmpp,
            tc.tile_pool(name="ystage", bufs=3) as ystage,
            tc.tile_pool(name="pps", bufs=4, space="PSUM") as pps,
            tc.tile_pool(name="sps", bufs=2, space="PSUM") as sps,
        ):
            wa_t = constp.tile([128, 3 * C], bf16, tag="wa")
            wb_t = constp.tile([128, C], bf16, tag="wb")
            bias_t = constp.tile([128, C], f32, tag="bias")
            nc.sync.dma_start(wa_t[:], wa_d[:, :])
            nc.sync.dma_start(wb_t[:], wb_d[:, :])
            nc.sync.dma_start(bias_t[:], bias_d[:, :])

            xs_t = [xsp.tile([128, N], bf16, tag=f"xs{s}", name=f"xs{s}")
                    for s in range(T)]
            at_t = [atp.tile([128, N], bf16, tag=f"at{mi}", name=f"at{mi}")
                    for mi in range(NB)]
            # DMA order: group-0 x slices first, then A^T (needed by spmm2),
            # then group-1 x slices.
            for s in range(G):
                nc.sync.dma_start(xs_t[s][:], xs_d[s, :, :])
            for mi in range(NB):
                nc.sync.dma_start(at_t[mi][:], at_d[mi, :, :])
            for s in range(G, T):
                nc.sync.dma_start(xs_t[s][:], xs_d[s, :, :])

            for g in range(NG):
                s0 = g * G
                p0_t = [p0p.tile([128, GW], mybir.dt.float32, tag=f"p0_{k}",
                                 name=f"p0_{g}_{k}") for k in range(NB)]
                p1_t = [grp.tile([128, GW], bf16, tag=f"p1_{k}",
                                 name=f"p1_{g}_{k}") for k in range(NB)]
                p2_t = [grp.tile([128, GW], bf16, tag=f"p2_{k}",
                                 name=f"p2_{g}_{k}") for k in range(NB)]
                m_t = [grp.tile([128, GW], bf16, tag=f"m_{k}",
                                name=f"m_{g}_{k}") for k in range(NB)]

                # --- projections: P1 | P2 | P0 per (slice, node-block) ---
                for idx in range(G):
                    s = s0 + idx
                    cs = slice(idx * C, (idx + 1) * C)
                    for k in range(NB):
                        pp = pps.tile([128, 3 * C], mybir.dt.float32, tag="pp")
                        lhsT = xs_t[s][:, k * 128:(k + 1) * 128]
                        nc.tensor.matmul(pp[:, 0:3 * C], lhsT, wa_t[:],
                                         start=True, stop=False)
                        nc.tensor.matmul(pp[:, 2 * C:3 * C], lhsT, wb_t[:],
                                         start=False, stop=True)
                        nc.vector.tensor_copy(p1_t[k][:, cs], pp[:, 0:C])
                        nc.vector.tensor_copy(p2_t[k][:, cs], pp[:, C:2 * C])
                        nc.vector.tensor_tensor(p0_t[k][:, cs], pp[:, 2 * C:3 * C],
                                                bias_t[:],
                                                op=mybir.AluOpType.add)

                # --- spmm2: Q = A@P2 ; M = P1 - 2Q ---
                for k in range(NB):
                    sp = sps.tile([128, GW], mybir.dt.float32, tag="sp")
                    for mi in range(NB):
                        nc.tensor.matmul(sp[:], at_t[mi][:, k * 128:(k + 1) * 128],
                                         p2_t[mi][:],
                                         start=(mi == 0), stop=(mi == NB - 1))
                    mt = mtmpp.tile([128, GW], bf16, tag="mtmp")
                    nc.scalar.mul(mt[:], sp[:], -2.0)
                    nc.vector.tensor_add(m_t[k][:], mt[:], p1_t[k][:])

                # --- spmm3: S = A@M ; y = P0 - S ---
                for k in range(NB):
                    sp = sps.tile([128, GW], mybir.dt.float32, tag="sp")
                    for mi in range(NB):
                        nc.tensor.matmul(sp[:], at_t[mi][:, k * 128:(k + 1) * 128],
                                         m_t[mi][:],
                                         start=(mi == 0), stop=(mi == NB - 1))
                    yt = ystage.tile([128, GW], mybir.dt.float32, tag="y")
                    nc.vector.tensor_sub(yt[:], p0_t[k][:], sp[:])
                    dst = y_d[s0:s0 + G, k * 128:(k + 1) * 128, :]
                    dst = dst.rearrange("s n c -> n s c")
                    nc.sync.dma_start(dst, yt[:])

    nc.compile()
    _NC_CACHE["nc"] = nc
    return nc


def _prep_inputs(x, A_norm, weight, bias):
    """Host-side shard + layout prep. Returns per-core input maps."""
    x = np.asarray(x, dtype=np.float32)
    A_norm = np.asarray(A_norm, dtype=np.float32)
    weight = np.asarray(weight, dtype=np.float32)
    bias = np.asarray(bias, dtype=np.float32)

    # A^T tiled by contraction block: at[mi, p, n] = A[n, mi*128+p]
    at_host = np.ascontiguousarray(A_norm.T).reshape(NB, 128, N).astype(BF16)

    W0, W1, W2 = weight[0], weight[1], weight[2]
    W02 = W0 - W2
    W02hi = W02.astype(BF16)
    W02lo = (W02 - W02hi.astype(np.float32)).astype(BF16)
    W1b = W1.astype(BF16)
    W2b = W2.astype(BF16)
    wa_host = np.zeros((128, 3 * C), dtype=BF16)
    wa_host[0:C, 0:C] = W1b
    wa_host[C:2 * C, 0:C] = W1b
    wa_host[0:C, C:2 * C] = W2b
    wa_host[C:2 * C, C:2 * C] = W2b
    wa_host[0:C, 2 * C:3 * C] = W02hi
    wa_host[C:2 * C, 2 * C:3 * C] = W02hi
    wb_host = np.zeros((128, C), dtype=BF16)
    wb_host[0:C, :] = W02lo

    bias_host = np.ascontiguousarray(np.broadcast_to(bias, (128, C)),
                                     dtype=np.float32)

    in_maps = []
    for b in range(B):
        xt = np.ascontiguousarray(x[b].transpose(0, 2, 1))  # [T, C, N]
        hi = xt.astype(BF16)
        lo = (xt - hi.astype(np.float32)).astype(BF16)
        xs_host = np.concatenate([hi, lo], axis=1)          # [T, 128, N]
        in_maps.append({
            "at": at_host,
            "xs": np.ascontiguousarray(xs_host),
            "wa": wa_host,
            "wb": wb_host,
            "biasb": bias_host,
        })
    return in_maps


def kernel(x, A_norm, weight, bias):
    nc = _build_nc()
    in_maps = _prep_inputs(x, A_norm, weight, bias)
    res = bass_utils.run_bass_kernel_spmd(nc, in_maps, core_ids=list(range(8)))
    out = np.stack([res.results[b]["y"] for b in range(B)], axis=0)
    return out.astype(np.float32)


# revision 10
# speedup vs baseline: 3.0556x; 3.0556x over previous
"""ChebGraphConv (K=3) Trainium2 kernel.

y = x@(W0-W2) - (A@x)@W1 + 2*A@((A@x)@W2) + bias

computed per (b,t) slice as:
  P0 = X@W02 + bias ; P1 = X@W1 ; P2 = X@W2   (projections, from X^T hi/lo bf16)
  Q = A@P2 ; M = P1 - 2Q ; S = A@M ; y = P0 - S  (spmms, bf16 A^T stationary)

Data parallel over B: core i handles x[i] (T=12 slices), slices processed in
groups of G=6 so the spmm moving operand is [128, 384].
"""

import numpy as np
import ml_dtypes

import concourse.bacc as bacc
import concourse.mybir as mybir
import concourse.tile as tile
from concourse import bass_utils

BF16 = ml_dtypes.bfloat16

B, T, N, C = 8, 12, 2048, 64
NB = N // 128          # 16 node blocks
G = 6                  # slices per group
NG = T // G            # 2 groups
GW = G * C             # 384: group free width

_NC_CACHE = {}


def _build_nc(repeat=None):
    """repeat=None: single-shot kernel (graded path). repeat=R: wraps the
    whole body in a hardware For loop running it R times (benchmarking)."""
    key = ("nc", repeat)
    if key in _NC_CACHE:
        return _NC_CACHE[key]
    f32 = mybir.dt.float32
    bf16 = mybir.dt.bfloat16

    nc = bacc.Bacc("TRN2", target_bir_lowering=False, debug=False,
                   enable_asserts=False, num_devices=8)

    at_d = nc.dram_tensor("at", [NB, 128, N], bf16, kind="ExternalInput")
    xs_d = nc.dram_tensor("xs", [T, 128, N], bf16, kind="ExternalInput")
    wa_d = nc.dram_tensor("wa", [128, 3 * C], bf16, kind="ExternalInput")
    wb_d = nc.dram_tensor("wb", [128, C], bf16, kind="ExternalInput")
    bias_d = nc.dram_tensor("biasb", [128, C], f32, kind="ExternalInput")
    y_d = nc.dram_tensor("y", [T, N, C], f32, kind="ExternalOutput")

    with tile.TileContext(nc) as tc:
        with (
            tc.tile_pool(name="const", bufs=1) as constp,
            tc.tile_pool(name="atp", bufs=1) as atp,
            tc.tile_pool(name="xsp", bufs=1) as xsp,
            tc.tile_pool(name="p0p", bufs=1) as p0p,
            tc.tile_pool(name="grp", bufs=1) as grp,
            tc.tile_pool(name="mtmpp", bufs=2) as mtmpp,
            tc.tile_pool(name="ystage", bufs=3) as ystage,
            tc.tile_pool(name="pps", bufs=4, space="PSUM") as pps,
            tc.tile_pool(name="sps", bufs=3, space="PSUM") as sps,
        ):
            def emit_body():
                _emit(nc, constp, atp, xsp, p0p, grp, mtmpp, ystage, pps, sps,
                      at_d, xs_d, wa_d, wb_d, bias_d, y_d)

            if repeat is None:
                emit_body()
            else:
                with tc.For_i(0, repeat, 1):
                    emit_body()

    nc.compile()
    _NC_CACHE[key] = nc
    return nc


def _emit(nc, constp, atp, xsp, p0p, grp, mtmpp, ystage, pps, sps,
          at_d, xs_d, wa_d, wb_d, bias_d, y_d):
    f32 = mybir.dt.float32
    bf16 = mybir.dt.bfloat16
    if True:
        if True:
            wa_t = constp.tile([128, 3 * C], bf16, tag="wa")
            wb_t = constp.tile([128, C], bf16, tag="wb")
            bias_t = constp.tile([128, C], f32, tag="bias")
            nc.sync.dma_start(wa_t[:], wa_d[:, :])
            nc.sync.dma_start(wb_t[:], wb_d[:, :])
            nc.sync.dma_start(bias_t[:], bias_d[:, :])

            xs_t = [xsp.tile([128, N], bf16, tag=f"xs{s}", name=f"xs{s}")
                    for s in range(T)]
            at_t = [atp.tile([128, N], bf16, tag=f"at{mi}", name=f"at{mi}")
                    for mi in range(NB)]
            # DMA order: group-0 x slices first, then A^T (needed by spmm2),
            # then group-1 x slices.
            for s in range(G):
                nc.sync.dma_start(xs_t[s][:], xs_d[s, :, :])
            for mi in range(NB):
                nc.sync.dma_start(at_t[mi][:], at_d[mi, :, :])
            for s in range(G, T):
                nc.sync.dma_start(xs_t[s][:], xs_d[s, :, :])

            for g in range(NG):
                s0 = g * G
                p0_t = [p0p.tile([128, GW], mybir.dt.float32, tag=f"p0_{k}",
                                 name=f"p0_{g}_{k}") for k in range(NB)]
                # p1/p2 interleaved per slice: cols [idx*128, idx*128+64) = P1,
                # [idx*128+64, (idx+1)*128) = P2' (=X@(2*W2))
                p12_t = [grp.tile([128, 2 * GW], bf16, tag=f"p12_{k}",
                                  name=f"p12_{g}_{k}") for k in range(NB)]
                m_t = [grp.tile([128, GW], bf16, tag=f"m_{k}",
                                name=f"m_{g}_{k}") for k in range(NB)]
                p12_v = [t.rearrange("p (i c) -> p i c", c=128) for t in p12_t]

                # --- projections: P1 | P2' | P0 per (slice, node-block) ---
                for idx in range(G):
                    s = s0 + idx
                    cs = slice(idx * C, (idx + 1) * C)
                    for k in range(NB):
                        pp = pps.tile([128, 3 * C], mybir.dt.float32, tag="pp")
                        lhsT = xs_t[s][:, k * 128:(k + 1) * 128]
                        nc.tensor.matmul(pp[:, 0:3 * C], lhsT, wa_t[:],
                                         start=True, stop=False)
                        nc.tensor.matmul(pp[:, 2 * C:3 * C], lhsT, wb_t[:],
                                         start=False, stop=True)
                        nc.vector.tensor_copy(
                            p12_t[k][:, idx * 128:(idx + 1) * 128], pp[:, 0:2 * C])
                        nc.vector.tensor_tensor(p0_t[k][:, cs], pp[:, 2 * C:3 * C],
                                                bias_t[:],
                                                op=mybir.AluOpType.add)

                # --- spmm2: Q' = A@P2' ; M = P1 - Q' ---
                for k in range(NB):
                    sp = sps.tile([128, GW], mybir.dt.float32, tag="sp")
                    for mi in range(NB):
                        nc.tensor.matmul(sp[:], at_t[mi][:, k * 128:(k + 1) * 128],
                                         p12_v[mi][:, :, C:2 * C],
                                         start=(mi == 0), stop=(mi == NB - 1))
                    nc.vector.tensor_tensor(m_t[k][:], p12_v[k][:, :, 0:C], sp[:],
                                            op=mybir.AluOpType.subtract)

                # --- spmm3: S = A@M ; y = P0 - S ---
                for k in range(NB):
                    sp = sps.tile([128, GW], mybir.dt.float32, tag="sp")
                    for mi in range(NB):
                        nc.tensor.matmul(sp[:], at_t[mi][:, k * 128:(k + 1) * 128],
                                         m_t[mi][:],
                                         start=(mi == 0), stop=(mi == NB - 1))
                    yt = ystage.tile([128, GW], mybir.dt.float32, tag="y")
                    nc.vector.tensor_sub(yt[:], p0_t[k][:], sp[:])
                    dst = y_d[s0:s0 + G, k * 128:(k + 1) * 128, :]
                    dst = dst.rearrange("s n c -> n s c")
                    nc.sync.dma_start(dst, yt[:])


def _prep_inputs(x, A_norm, weight, bias):
    """Host-side shard + layout prep. Returns per-core input maps."""
    x = np.asarray(x, dtype=np.float32)
    A_norm = np.asarray(A_norm, dtype=np.float32)
    weight = np.asarray(weight, dtype=np.float32)
    bias = np.asarray(bias, dtype=np.float32)

    # A^T tiled by contraction block: at[mi, p, n] = A[n, mi*128+p]
    at_host = np.ascontiguousarray(A_norm.T).reshape(NB, 128, N).astype(BF16)

    W0, W1, W2 = weight[0], weight[1], weight[2]
    W02 = W0 - W2
    W02hi = W02.astype(BF16)
    W02lo = (W02 - W02hi.astype(np.float32)).astype(BF16)
    W1b = W1.astype(BF16)
    W2b = (2.0 * W2).astype(BF16)
    wa_host = np.zeros((128, 3 * C), dtype=BF16)
    wa_host[0:C, 0:C] = W1b
    wa_host[C:2 * C, 0:C] = W1b
    wa_host[0:C, C:2 * C] = W2b      # 2*W2: Q' = A@(X@2W2) so M = P1 - Q'
    wa_host[C:2 * C, C:2 * C] = W2b
    wa_host[0:C, 2 * C:3 * C] = W02hi
    wa_host[C:2 * C, 2 * C:3 * C] = W02hi
    wb_host = np.zeros((128, C), dtype=BF16)
    wb_host[0:C, :] = W02lo

    bias_host = np.ascontiguousarray(np.broadcast_to(bias, (128, C)),
                                     dtype=np.float32)

    in_maps = []
    for b in range(B):
        xt = np.ascontiguousarray(x[b].transpose(0, 2, 1))  # [T, C, N]
        hi = xt.astype(BF16)
        lo = (xt - hi.astype(np.float32)).astype(BF16)
        xs_host = np.concatenate([hi, lo], axis=1)          # [T, 128, N]
        in_maps.append({
            "at": at_host,
            "xs": np.ascontiguousarray(xs_host),
            "wa": wa_host,
            "wb": wb_host,
            "biasb": bias_host,
        })
    return in_maps


def kernel(x, A_norm, weight, bias):
    nc = _build_nc()
    in_maps = _prep_inputs(x, A_norm, weight, bias)
    res = bass_utils.run_bass_kernel_spmd(nc, in_maps, core_ids=list(range(8)))
    out = np.stack([res.results[b]["y"] for b in range(B)], axis=0)
    return out.astype(np.float32)


# revision 11
# speedup vs baseline: 3.1149x; 1.0194x over previous
"""ChebGraphConv (K=3) Trainium2 kernel.

y = x@(W0-W2) - (A@x)@W1 + 2*A@((A@x)@W2) + bias

computed per (b,t) slice as:
  P0 = X@W02 + bias ; P1 = X@W1 ; P2 = X@W2   (projections, from X^T hi/lo bf16)
  Q = A@P2 ; M = P1 - 2Q ; S = A@M ; y = P0 - S  (spmms, bf16 A^T stationary)

Data parallel over B: core i handles x[i] (T=12 slices), slices processed in
groups of G=6 so the spmm moving operand is [128, 384].
"""

import numpy as np
import ml_dtypes

import concourse.bacc as bacc
import concourse.mybir as mybir
import concourse.tile as tile
from concourse import bass_utils

BF16 = ml_dtypes.bfloat16

B, T, N, C = 8, 12, 2048, 64
NB = N // 128          # 16 node blocks
G = 6                  # slices per group
NG = T // G            # 2 groups
GW = G * C             # 384: group free width

_NC_CACHE = {}


def _build_nc(repeat=None):
    """repeat=None: single-shot kernel (graded path). repeat=R: wraps the
    whole body in a hardware For loop running it R times (benchmarking)."""
    key = ("nc", repeat)
    if key in _NC_CACHE:
        return _NC_CACHE[key]
    f32 = mybir.dt.float32
    bf16 = mybir.dt.bfloat16

    nc = bacc.Bacc("TRN2", target_bir_lowering=False, debug=False,
                   enable_asserts=False, num_devices=8)

    at_d = nc.dram_tensor("at", [NB, 128, N], bf16, kind="ExternalInput")
    xs_d = nc.dram_tensor("xs", [T, 128, N], bf16, kind="ExternalInput")
    wa_d = nc.dram_tensor("wa", [128, 3 * C], bf16, kind="ExternalInput")
    wb_d = nc.dram_tensor("wb", [128, C], bf16, kind="ExternalInput")
    bias_d = nc.dram_tensor("biasb", [128, C], f32, kind="ExternalInput")
    y_d = nc.dram_tensor("y", [T, N, C], f32, kind="ExternalOutput")

    with tile.TileContext(nc) as tc:
        with (
            tc.tile_pool(name="const", bufs=1) as constp,
            tc.tile_pool(name="atp", bufs=1) as atp,
            tc.tile_pool(name="xsp", bufs=1) as xsp,
            tc.tile_pool(name="p0p", bufs=1) as p0p,
            tc.tile_pool(name="grp", bufs=1) as grp,
            tc.tile_pool(name="mtmpp", bufs=2) as mtmpp,
            tc.tile_pool(name="ystage", bufs=3) as ystage,
            tc.tile_pool(name="pps", bufs=4, space="PSUM") as pps,
            tc.tile_pool(name="sps", bufs=3, space="PSUM") as sps,
        ):
            def emit_body():
                _emit(nc, constp, atp, xsp, p0p, grp, mtmpp, ystage, pps, sps,
                      at_d, xs_d, wa_d, wb_d, bias_d, y_d)

            if repeat is None:
                emit_body()
            else:
                with tc.For_i(0, repeat, 1):
                    emit_body()

    nc.compile()
    _NC_CACHE[key] = nc
    return nc


def _emit(nc, constp, atp, xsp, p0p, grp, mtmpp, ystage, pps, sps,
          at_d, xs_d, wa_d, wb_d, bias_d, y_d):
    f32 = mybir.dt.float32
    bf16 = mybir.dt.bfloat16
    if True:
        if True:
            wa_t = constp.tile([128, 3 * C], bf16, tag="wa")
            wb_t = constp.tile([128, C], bf16, tag="wb")
            bias_t = constp.tile([128, C], f32, tag="bias")
            nc.sync.dma_start(wa_t[:], wa_d[:, :])
            nc.sync.dma_start(wb_t[:], wb_d[:, :])
            nc.sync.dma_start(bias_t[:], bias_d[:, :])

            xs_t = [xsp.tile([128, N], bf16, tag=f"xs{s}", name=f"xs{s}")
                    for s in range(T)]
            at_t = [atp.tile([128, N], bf16, tag=f"at{mi}", name=f"at{mi}")
                    for mi in range(NB)]
            # DMA order: group-0 x slices first, then A^T (needed by spmm2),
            # then group-1 x slices.
            for s in range(G):
                nc.sync.dma_start(xs_t[s][:], xs_d[s, :, :])
            for mi in range(NB):
                nc.sync.dma_start(at_t[mi][:], at_d[mi, :, :])
            for s in range(G, T):
                nc.sync.dma_start(xs_t[s][:], xs_d[s, :, :])

            for g in range(NG):
                s0 = g * G
                p0_t = [p0p.tile([128, GW], mybir.dt.float32, tag=f"p0_{k}",
                                 name=f"p0_{g}_{k}") for k in range(NB)]
                p1_t = [grp.tile([128, GW], bf16, tag=f"p1_{k}",
                                 name=f"p1_{g}_{k}") for k in range(NB)]
                p2_t = [grp.tile([128, GW], bf16, tag=f"p2_{k}",
                                 name=f"p2_{g}_{k}") for k in range(NB)]
                m_t = [grp.tile([128, GW], bf16, tag=f"m_{k}",
                                name=f"m_{g}_{k}") for k in range(NB)]

                # --- projections: P1 | P2' | P0 per (slice, node-block) ---
                for idx in range(G):
                    s = s0 + idx
                    cs = slice(idx * C, (idx + 1) * C)
                    for k in range(NB):
                        pp = pps.tile([128, 3 * C], mybir.dt.float32, tag="pp")
                        lhsT = xs_t[s][:, k * 128:(k + 1) * 128]
                        nc.tensor.matmul(pp[:, 0:3 * C], lhsT, wa_t[:],
                                         start=True, stop=False)
                        nc.tensor.matmul(pp[:, 2 * C:3 * C], lhsT, wb_t[:],
                                         start=False, stop=True)
                        nc.vector.tensor_copy(p1_t[k][:, cs], pp[:, 0:C])
                        nc.vector.tensor_copy(p2_t[k][:, cs], pp[:, C:2 * C])
                        nc.vector.tensor_tensor(p0_t[k][:, cs], pp[:, 2 * C:3 * C],
                                                bias_t[:],
                                                op=mybir.AluOpType.add)

                # --- spmm2: Q' = A@P2' ; M = P1 - Q' ---
                for k in range(NB):
                    sp = sps.tile([128, GW], mybir.dt.float32, tag="sp")
                    for mi in range(NB):
                        nc.tensor.matmul(sp[:], at_t[mi][:, k * 128:(k + 1) * 128],
                                         p2_t[mi][:],
                                         start=(mi == 0), stop=(mi == NB - 1))
                    nc.vector.tensor_tensor(m_t[k][:], p1_t[k][:], sp[:],
                                            op=mybir.AluOpType.subtract)

                # --- spmm3: S = A@M ; y = P0 - S ---
                for k in range(NB):
                    sp = sps.tile([128, GW], mybir.dt.float32, tag="sp")
                    for mi in range(NB):
                        nc.tensor.matmul(sp[:], at_t[mi][:, k * 128:(k + 1) * 128],
                                         m_t[mi][:],
                                         start=(mi == 0), stop=(mi == NB - 1))
                    yt = ystage.tile([128, GW], mybir.dt.float32, tag="y")
                    nc.vector.tensor_sub(yt[:], p0_t[k][:], sp[:])
                    dst = y_d[s0:s0 + G, k * 128:(k + 1) * 128, :]
                    dst = dst.rearrange("s n c -> n s c")
                    nc.sync.dma_start(dst, yt[:])


def _prep_inputs(x, A_norm, weight, bias):
    """Host-side shard + layout prep. Returns per-core input maps."""
    x = np.asarray(x, dtype=np.float32)
    A_norm = np.asarray(A_norm, dtype=np.float32)
    weight = np.asarray(weight, dtype=np.float32)
    bias = np.asarray(bias, dtype=np.float32)

    # A^T tiled by contraction block: at[mi, p, n] = A[n, mi*128+p]
    at_host = np.ascontiguousarray(A_norm.T).reshape(NB, 128, N).astype(BF16)

    W0, W1, W2 = weight[0], weight[1], weight[2]
    W02 = W0 - W2
    W02hi = W02.astype(BF16)
    W02lo = (W02 - W02hi.astype(np.float32)).astype(BF16)
    W1b = W1.astype(BF16)
    W2b = (2.0 * W2).astype(BF16)
    wa_host = np.zeros((128, 3 * C), dtype=BF16)
    wa_host[0:C, 0:C] = W1b
    wa_host[C:2 * C, 0:C] = W1b
    wa_host[0:C, C:2 * C] = W2b      # 2*W2: Q' = A@(X@2W2) so M = P1 - Q'
    wa_host[C:2 * C, C:2 * C] = W2b
    wa_host[0:C, 2 * C:3 * C] = W02hi
    wa_host[C:2 * C, 2 * C:3 * C] = W02hi
    wb_host = np.zeros((128, C), dtype=BF16)
    wb_host[0:C, :] = W02lo

    bias_host = np.ascontiguousarray(np.broadcast_to(bias, (128, C)),
                                     dtype=np.float32)

    in_maps = []
    for b in range(B):
        xt = np.ascontiguousarray(x[b].transpose(0, 2, 1))  # [T, C, N]
        hi = xt.astype(BF16)
        lo = (xt - hi.astype(np.float32)).astype(BF16)
        xs_host = np.concatenate([hi, lo], axis=1)          # [T, 128, N]
        in_maps.append({
            "at": at_host,
            "xs": np.ascontiguousarray(xs_host),
            "wa": wa_host,
            "wb": wb_host,
            "biasb": bias_host,
        })
    return in_maps


def kernel(x, A_norm, weight, bias):
    nc = _build_nc()
    in_maps = _prep_inputs(x, A_norm, weight, bias)
    res = bass_utils.run_bass_kernel_spmd(nc, in_maps, core_ids=list(range(8)))
    out = np.stack([res.results[b]["y"] for b in range(B)], axis=0)
    return out.astype(np.float32)


# revision 13
# speedup vs baseline: 3.2978x; 1.0587x over previous
"""ChebGraphConv (K=3) Trainium2 kernel.

y = x@(W0-W2) - (A@x)@W1 + 2*A@((A@x)@W2) + bias

computed per (b,t) slice as:
  P0 = X@W02 ; P1 = X@W1 ; P2' = X@(2*W2)   (projections from X^T hi/lo bf16,
                                             one 256-wide matmul per node block)
  Q' = A@P2' ; M = P1 - Q' ; S = A@M ; y = P0 - S (+bias)   (bf16 spmms,
                                             A^T tiles stationary)

Data parallel over B: core b handles x[b] (T=12 slices), slices processed in
groups of G=6 so the spmm moving operand is [128, 384].

All bf16 rounding lands on the small A@(...) terms (|A@v| ~ 0.01*|v|); the
dominant P0 term uses an X-hi/lo + W02-hi/lo split, so overall output error
stays ~3e-5 relative.
"""

import numpy as np
import ml_dtypes

import concourse.bacc as bacc
import concourse.mybir as mybir
import concourse.tile as tile
from concourse import bass_utils

BF16 = ml_dtypes.bfloat16

B, T, N, C = 8, 12, 2048, 64
NB = N // 128          # 16 node blocks
GROUPS = (6, 6)        # slices per group

_NC_CACHE = {}


def _build_nc(repeat=None, with_bias=False):
    """repeat=None: single-shot kernel (graded path). repeat=R: wraps the
    whole body in a hardware For loop running it R times (benchmarking)."""
    key = ("nc", repeat, with_bias)
    if key in _NC_CACHE:
        return _NC_CACHE[key]
    f32 = mybir.dt.float32
    bf16 = mybir.dt.bfloat16

    nc = bacc.Bacc("TRN2", target_bir_lowering=False, debug=False,
                   enable_asserts=False, num_devices=8)

    at_d = nc.dram_tensor("at", [NB, 128, N], bf16, kind="ExternalInput")
    xs_d = nc.dram_tensor("xs", [T, 128, N], bf16, kind="ExternalInput")
    wa_d = nc.dram_tensor("wa", [128, 4 * C], bf16, kind="ExternalInput")
    bias_d = nc.dram_tensor("biasb", [128, C], f32, kind="ExternalInput")
    y_d = nc.dram_tensor("y", [T, N, C], f32, kind="ExternalOutput")

    with tile.TileContext(nc) as tc:
        with (
            tc.tile_pool(name="const", bufs=1) as constp,
            tc.tile_pool(name="atp", bufs=1) as atp,
            tc.tile_pool(name="xsp", bufs=1) as xsp,
            tc.tile_pool(name="bigp", bufs=1) as bigp,
            tc.tile_pool(name="ystage", bufs=3) as ystage,
            tc.tile_pool(name="pps", bufs=4, space="PSUM") as pps,
            tc.tile_pool(name="sps", bufs=3, space="PSUM") as sps,
        ):
            def emit_body():
                _emit(nc, constp, atp, xsp, bigp, ystage, pps, sps,
                      at_d, xs_d, wa_d, bias_d, y_d, with_bias)

            if repeat is None:
                emit_body()
            else:
                with tc.For_i(0, repeat, 1):
                    emit_body()

    nc.compile()
    _NC_CACHE[key] = nc
    return nc


def _emit(nc, constp, atp, xsp, bigp, ystage, pps, sps,
          at_d, xs_d, wa_d, bias_d, y_d, with_bias):
    f32 = mybir.dt.float32
    bf16 = mybir.dt.bfloat16

    wa_t = constp.tile([128, 4 * C], bf16, tag="wa")
    bias_t = constp.tile([128, C], f32, tag="bias")
    nc.sync.dma_start(wa_t[:], wa_d[:, :])
    nc.sync.dma_start(bias_t[:], bias_d[:, :])

    xs_t = [xsp.tile([128, N], bf16, tag=f"xs{s}", name=f"xs{s}")
            for s in range(T)]
    at_t = [atp.tile([128, N], bf16, tag=f"at{mi}", name=f"at{mi}")
            for mi in range(NB)]
    # DMA order: group-0 x slices first, then A^T (needed from spmm2 on),
    # then the remaining x slices.
    for s in range(GROUPS[0]):
        nc.sync.dma_start(xs_t[s][:], xs_d[s, :, :])
    for mi in range(NB):
        nc.sync.dma_start(at_t[mi][:], at_d[mi, :, :])
    for s in range(GROUPS[0], T):
        nc.sync.dma_start(xs_t[s][:], xs_d[s, :, :])

    s0 = 0
    for g, G in enumerate(GROUPS):
        GW = G * C
        # big flat per-group tiles; p12 plane 0 = P1, plane 1 = P2'
        p12 = bigp.tile([128, 2, NB, GW], bf16, tag="p12", name=f"p12_{g}")
        p0 = bigp.tile([128, NB, GW], f32, tag="p0", name=f"p0_{g}")
        m = bigp.tile([128, NB, GW], bf16, tag="m", name=f"m_{g}")

        # --- projections: one [128,256] matmul per (slice, node block);
        #     two node blocks share one PSUM bank; no PSUM accumulation ---
        for idx in range(G):
            s = s0 + idx
            cs = slice(idx * C, (idx + 1) * C)
            for kp in range(NB // 2):
                pp = pps.tile([128, 512], f32, tag="pp", name="pp")
                for j in range(2):
                    k = 2 * kp + j
                    nc.tensor.matmul(pp[:, j * 256:(j + 1) * 256],
                                     xs_t[s][:, k * 128:(k + 1) * 128],
                                     wa_t[:], start=True, stop=True)
                # cols = k2*256 + pl*64 + c: pl 0=P1, 1=P2', 2=P0hi, 3=P0lo
                pv = pp.rearrange("p (k2 pl c) -> p pl k2 c", k2=2, pl=4, c=C)
                nc.vector.tensor_copy(p12[:, 0:2, 2 * kp:2 * kp + 2, cs],
                                      pv[:, 0:2, :, :])
                # two PSUM operands in one op are rejected by walrus:
                # copy hi, then accumulate lo
                p0sl = p0[:, 2 * kp:2 * kp + 2, cs]
                nc.vector.tensor_copy(p0sl, pv[:, 2, :, :])
                nc.vector.tensor_tensor(p0sl, p0sl, pv[:, 3, :, :],
                                        op=mybir.AluOpType.add)

        # --- spmm2: Q' = A@P2' ; M = P1 - Q' ---
        for k in range(NB):
            sp = sps.tile([128, GW], f32, tag="sp", name="sp")
            for mi in range(NB):
                nc.tensor.matmul(sp[:], at_t[mi][:, k * 128:(k + 1) * 128],
                                 p12[:, 1, mi, :],
                                 start=(mi == 0), stop=(mi == NB - 1))
            nc.vector.tensor_tensor(m[:, k, :], p12[:, 0, k, :], sp[:],
                                    op=mybir.AluOpType.subtract)

        # --- spmm3: S = A@M ; y = P0 - S (+bias) ---
        for k in range(NB):
            sp = sps.tile([128, GW], f32, tag="sp", name="sp")
            for mi in range(NB):
                nc.tensor.matmul(sp[:], at_t[mi][:, k * 128:(k + 1) * 128],
                                 m[:, mi, :],
                                 start=(mi == 0), stop=(mi == NB - 1))
            yt = ystage.tile([128, GW], f32, tag="y", name="yt")
            nc.vector.tensor_sub(yt[:], p0[:, k, :], sp[:])
            if with_bias:
                for idx in range(G):
                    ysl = yt[:, idx * C:(idx + 1) * C]
                    nc.vector.tensor_tensor(ysl, ysl, bias_t[:],
                                            op=mybir.AluOpType.add)
            dst = y_d[s0:s0 + G, k * 128:(k + 1) * 128, :]
            dst = dst.rearrange("s n c -> n s c")
            nc.sync.dma_start(dst, yt[:])
        s0 += G


def _prep_inputs(x, A_norm, weight, bias):
    """Host-side shard + layout prep. Returns per-core input maps."""
    x = np.asarray(x, dtype=np.float32)
    A_norm = np.asarray(A_norm, dtype=np.float32)
    weight = np.asarray(weight, dtype=np.float32)
    bias = np.asarray(bias, dtype=np.float32)

    # A^T tiled by contraction block: at[mi, p, n] = A[n, mi*128+p]
    at_host = np.ascontiguousarray(A_norm.T).reshape(NB, 128, N).astype(BF16)

    W0, W1, W2 = weight[0], weight[1], weight[2]
    W02 = W0 - W2
    W02hi = W02.astype(BF16)
    W02lo = (W02 - W02hi.astype(np.float32)).astype(BF16)
    W1b = W1.astype(BF16)
    W2b = (2.0 * W2).astype(BF16)  # fold the Chebyshev 2x into W2
    # wa columns: [W1 | 2*W2 | W02hi | W02lo]; rows 0:64 hit Xhi, 64:128 Xlo
    wa_host = np.zeros((128, 4 * C), dtype=BF16)
    wa_host[0:C, 0:C] = W1b
    wa_host[C:2 * C, 0:C] = W1b
    wa_host[0:C, C:2 * C] = W2b
    wa_host[C:2 * C, C:2 * C] = W2b
    wa_host[0:C, 2 * C:3 * C] = W02hi
    wa_host[C:2 * C, 2 * C:3 * C] = W02hi
    wa_host[0:C, 3 * C:4 * C] = W02lo

    bias_host = np.ascontiguousarray(np.broadcast_to(bias, (128, C)),
                                     dtype=np.float32)

    in_maps = []
    for b in range(B):
        xt = np.ascontiguousarray(x[b].transpose(0, 2, 1))  # [T, C, N]
        hi = xt.astype(BF16)
        lo = (xt - hi.astype(np.float32)).astype(BF16)
        xs_host = np.concatenate([hi, lo], axis=1)          # [T, 128, N]
        in_maps.append({
            "at": at_host,
            "xs": np.ascontiguousarray(xs_host),
            "wa": wa_host,
            "biasb": bias_host,
        })
    return in_maps


def kernel(x, A_norm, weight, bias):
    with_bias = bool(np.any(np.asarray(bias)))
    nc = _build_nc(with_bias=with_bias)
    in_maps = _prep_inputs(x, A_norm, weight, bias)
    res = bass_utils.run_bass_kernel_spmd(nc, in_maps, core_ids=list(range(8)))
    out = np.stack([res.results[b]["y"] for b in range(B)], axis=0)
    return out.astype(np.float32)
